# revision 10
# baseline (speedup 1.0000x reference)
"""DepthwiseSeparableAttention Trainium2 kernel (8-core SPMD).

Sharding: core c -> (batch b = c//4, head-group g = c%4, 4 heads each).
Each core computes depthwise-conv + QKV projection for its head slice,
attention for its 4 heads, and a partial output projection; the host sums
the 4 partials per batch and adds the output bias.

Schedule (v2): the scalar-engine exp stream (128 x [128,1024] EXPs ~= 128us)
is the critical resource.  The front is organised to start it ASAP (~50us):
x d-tile 0 ships first, the conv is split across DVE/GPSIMD/Act, q/k
projections use 2048-wide compound matmuls (q then k sequentially to fit
PSUM).  v conv + v projection, attention-V (fp8 DoubleRow over persisted
fp8 p slabs), softmax normalization and the output projection all run
under / after the exp stream, threaded into its windows so the Act engine
never stalls.
"""
import os
import sys
for _p in ('/opt/trn_rl_repo', '/root/.axon_site/_ro/trn_rl_repo'):
    if os.path.isdir(_p):
        sys.path.insert(0, _p)
        break

import numpy as np
import ml_dtypes

import concourse.bass as bass
import concourse.mybir as mybir
import concourse.tile as tile
from concourse.vector_clock import ScopedClock

BF16 = mybir.dt.bfloat16
F32 = mybir.dt.float32
FP8 = mybir.dt.float8e4
AF = mybir.ActivationFunctionType
ALU = mybir.AluOpType
DR = mybir.MatmulPerfMode.DoubleRow

S = 2048          # sequence length
D = 1024          # model dim
DT = 8            # d-tiles of 128
JL = 256          # local head channels (4 heads x 64)
HS = 80           # per-head stride in vx (64 d + ones col + pad; 16B-aligned)
N_CORES = 8

# ---------------------------------------------------------------------------
# walrus in this env allows only ONE sync wait per instruction; split Tile's
# excess waits onto no-fuse NOPs / extra drains.
MAX_WAITS = 1


def _patched_drain_and_barrier(self, tick_clock, wait_clock):
    drain_inst = self.nc.sync.drain()
    wait_clock.add_sem_waits(drain_inst.ins, ScopedClock({None: tick_clock.global_clock}))
    si = drain_inst.ins.sync_info
    if si is not None and len(si.on_wait) > 1:
        waits = list(si.on_wait)
        drain_inst.ins.sync_info = mybir.SyncInfo(on_wait=[waits[0]], on_update=list(si.on_update))
        for w in waits[1:]:
            d2 = self.nc.sync.drain()
            d2.ins.sync_info = mybir.SyncInfo(on_wait=[w], on_update=[])
    self.nc.all_engine_barrier()
    popped = self.nc._tile_sem_poison_stack.pop()
    assert popped is self._sem_poison
    self.nc.clear_and_free_semaphores(list(self.sems.allocated().values()))
    self.nc.all_engine_barrier()


tile.TileContext._drain_and_barrier = _patched_drain_and_barrier


def split_multi_waits(nc):
    n_split = 0
    for f in nc.m.functions:
        for blk in f.blocks:
            il = blk.instructions
            if not any(i.sync_info and len(i.sync_info.on_wait) > MAX_WAITS for i in il):
                continue
            newlist = []
            for inst in il:
                si = inst.sync_info
                if si is not None and len(si.on_wait) > MAX_WAITS:
                    waits = list(si.on_wait)
                    head, tail = waits[:-MAX_WAITS], waits[-MAX_WAITS:]
                    for j, w in enumerate(head):
                        nop = mybir.InstNoOp(
                            name=f"{inst.name}-w{j}",
                            sync_info=mybir.SyncInfo(on_wait=[w], on_update=[]),
                            bass_nofuse=True,
                            engine=inst.engine,
                        )
                        newlist.append(nop)
                        n_split += 1
                    inst.sync_info = mybir.SyncInfo(on_wait=tail, on_update=list(si.on_update))
                newlist.append(inst)
            blk.instructions = newlist
    return n_split


# ---------------------------------------------------------------------------
def build_program(n_rep=1):
    import contextlib
    nc = bass.Bass()
    P = {}
    P['xpE'] = nc.declare_dram_parameter("xpE", [128, DT, S + 4], BF16, isOutput=False)
    P['xpO'] = nc.declare_dram_parameter("xpO", [128, DT, S + 4], BF16, isOutput=False)
    for t in ("q", "k", "v"):
        P['w' + t] = nc.declare_dram_parameter("w" + t, [128, DT, JL], BF16, isOutput=False)
        P['tap' + t] = nc.declare_dram_parameter("tap" + t, [128, DT, 3], F32, isOutput=False)
        P['cb' + t] = nc.declare_dram_parameter("cb" + t, [128, DT], F32, isOutput=False)
    P['pbq'] = nc.declare_dram_parameter("pbq", [128, 2], F32, isOutput=False)
    P['pbk'] = nc.declare_dram_parameter("pbk", [128, 2], F32, isOutput=False)
    P['bv2'] = nc.declare_dram_parameter("bv2", [1, JL], BF16, isOutput=False)
    P['wo'] = nc.declare_dram_parameter("wo", [128, 2, D], BF16, isOutput=False)
    P['y'] = nc.declare_dram_parameter("y", [D, S], F32, isOutput=True)
    rdram2 = nc.dram_tensor("recip_scratch2", [16, 512], F32)

    with tile.TileContext(nc) as tc:
        with contextlib.ExitStack() as ctx:
            # outer pools: live for the whole kernel
            consts = ctx.enter_context(tc.tile_pool(name="consts", bufs=1))
            qkvp = ctx.enter_context(tc.tile_pool(name="qkvp", bufs=1))
            cvpool = ctx.enter_context(tc.tile_pool(name="cvpool", bufs=1))
            bigpA = ctx.enter_context(tc.tile_pool(name="bigpA", bufs=2))
            nrmp = ctx.enter_context(tc.tile_pool(name="nrm", bufs=1))

            # scope-1: x + conv transients; closed mid-attention so the late
            # p tiles (bigpB) can reuse the SBUF.
            s1 = ctx.enter_context(contextlib.ExitStack())
            xpool = s1.enter_context(tc.tile_pool(name="xpool", bufs=1))

            # ---- constants + input, DMA'd in consumption order -------------
            tap_sb, cb_sb, w_sb, pb_sb = {}, {}, {}, {}
            for t in ("q", "k"):
                tap_sb[t] = consts.tile([128, DT, 3], F32, name="tap_" + t)
                nc.sync.dma_start(out=tap_sb[t][:], in_=P['tap' + t][:])
                cb_sb[t] = consts.tile([128, DT], F32, name="cb_" + t)
                nc.sync.dma_start(out=cb_sb[t][:], in_=P['cb' + t][:])
            xpE = xpool.tile([128, DT, S + 4], BF16, name="xpE")
            xpO = xpool.tile([128, DT, S + 4], BF16, name="xpO")
            nc.sync.dma_start(out=xpE[:, 0, :], in_=P['xpE'][:, 0, :])
            nc.sync.dma_start(out=xpO[:, 0, :], in_=P['xpO'][:, 0, :])
            for t in ("q", "k"):
                w_sb[t] = consts.tile([128, DT, JL], BF16, name="w_" + t)
                nc.sync.dma_start(out=w_sb[t][:], in_=P['w' + t][:])
            nc.sync.dma_start(out=xpE[:, 1, :], in_=P['xpE'][:, 1, :])
            nc.sync.dma_start(out=xpO[:, 1, :], in_=P['xpO'][:, 1, :])
            for t in ("q", "k"):
                pb_sb[t] = consts.tile([128, 2], F32, name="pb_" + t)
                nc.sync.dma_start(out=pb_sb[t][:], in_=P['pb' + t][:])
            for d in range(2, DT):
                nc.sync.dma_start(out=xpE[:, d, :], in_=P['xpE'][:, d, :])
                nc.sync.dma_start(out=xpO[:, d, :], in_=P['xpO'][:, d, :])
            tap_sb["v"] = consts.tile([128, DT, 3], F32, name="tap_v")
            nc.sync.dma_start(out=tap_sb["v"][:], in_=P['tapv'][:])
            cb_sb["v"] = consts.tile([128, DT], F32, name="cb_v")
            nc.sync.dma_start(out=cb_sb["v"][:], in_=P['cbv'][:])
            w_sb["v"] = consts.tile([128, DT, JL], BF16, name="w_v")
            nc.sync.dma_start(out=w_sb["v"][:], in_=P['wv'][:])
            bv2_sb = consts.tile([1, JL], BF16)
            nc.sync.dma_start(out=bv2_sb[:], in_=P['bv2'][:])
            wo_sb = consts.tile([128, 2, D], BF16)
            nc.sync.dma_start(out=wo_sb[:], in_=P['wo'][:])
            ones_sb = consts.tile([1, 128], BF16)
            nc.vector.memset(ones_sb[:], 1.0)

            # ---- persistent activations -----------------------------------
            qT = qkvp.tile([128, 2, S], BF16, name="qT")      # [j_in_tile, j_tile, s]
            kT = qkvp.tile([128, 2, S], BF16, name="kT")
            # v in fp8, [s_in_tile, s_tile, head*HS]: col 64 of each head
            # block is the softmax-denominator ones column, 65..79 pad
            # (HS=80 keeps the DoubleRow weight k-tile step 16B-aligned).
            vx = qkvp.tile([128, 16, 4 * HS], FP8, name="vx")
            for h in range(4):
                nc.vector.memset(vx[:, :, HS * h + 64: HS * h + 65], 1.0)
                nc.vector.memset(vx[:, :, HS * h + 65: HS * (h + 1)], 0.0)
            attn_out = qkvp.tile([128, 8, 512], BF16, name="attn_out")

            # ================= phase B: conv + QK projections ================
            #   c1 = mid*x + bias (Act); t0 = left tap, c2 = right tap (DVE);
            #   t0 += c2; t0 += c1 (adds split DVE/GPSIMD); one 2048-wide
            #   compound matmul per (t, d, m).  q fully, then k (their two
            #   4-bank PSUM accumulator pairs don't fit at once).
            GP_TT1 = (2, 3, 5)   # d-tiles whose first add goes to GPSIMD
            GP_TT2 = (2, 5)      # ... and second add
            with tc.tile_pool(name="convqk", bufs=2) as convqk, \
                 tc.tile_pool(name="psqk", bufs=2, space=bass.MemorySpace.PSUM) as psqk:
                for t, dst in (("q", qT), ("k", kT)):
                    ps = [psqk.tile([128, S], F32, name="ps_qk") for _ in range(2)]
                    for d in range(DT):
                        c1 = convqk.tile([128, S], BF16, name="c1")
                        nc.scalar.activation(
                            c1[:], xpE[:, d, 2:S + 2], AF.Identity,
                            bias=cb_sb[t][:, d:d + 1], scale=tap_sb[t][:, d, 1:2])
                        t0 = convqk.tile([128, S], BF16, name="t0")
                        nc.vector.tensor_scalar(
                            out=t0[:], in0=xpO[:, d, 2:S + 2],
                            scalar1=tap_sb[t][:, d, 0:1], scalar2=None, op0=ALU.mult)
                        c2 = convqk.tile([128, S], BF16, name="c2")
                        nc.vector.tensor_scalar(
                            out=c2[:], in0=xpO[:, d, 4:S + 4],
                            scalar1=tap_sb[t][:, d, 2:3], scalar2=None, op0=ALU.mult)
                        eng1 = nc.gpsimd if d in GP_TT1 else nc.vector
                        eng1.tensor_tensor(out=t0[:], in0=t0[:], in1=c2[:], op=ALU.add)
                        eng2 = nc.gpsimd if d in GP_TT2 else nc.vector
                        eng2.tensor_tensor(out=t0[:], in0=t0[:], in1=c1[:], op=ALU.add)
                        for m in range(2):
                            for c in range(4):
                                nc.tensor.matmul(
                                    ps[m][:, 512 * c: 512 * (c + 1)],
                                    w_sb[t][:, d, 128 * m: 128 * (m + 1)],
                                    t0[:, 512 * c: 512 * (c + 1)],
                                    start=(d == 0), stop=(d == DT - 1))
                    if t == "q":
                        for m in range(2):
                            nc.scalar.activation(
                                dst[:, m, :], ps[m][:], AF.Identity,
                                bias=pb_sb[t][:, m: m + 1], scale=1.0)
                    else:
                        # m0 (pair 0) drains on Act so scores start ASAP;
                        # m1 goes to DVE to keep Act free for exp
                        # (GPSIMD cannot read PSUM).
                        nc.scalar.activation(
                            dst[:, 0, :], ps[0][:], AF.Identity,
                            bias=pb_sb[t][:, 0:1], scale=1.0)
                        nc.vector.tensor_scalar(
                            out=dst[:, 1, :], in0=ps[1][:],
                            scalar1=pb_sb[t][:, 1:2], scalar2=None, op0=ALU.add)

            # ================= phase C: attention (+ v under it) =============
            # PSUM: scores 2x2 banks + attn acc 2x1 + v-proj 2x1 = 8.
            s2 = ctx.enter_context(contextlib.ExitStack())
            scorep = s2.enter_context(
                tc.tile_pool(name="scores", bufs=2, space=bass.MemorySpace.PSUM))
            attnp = s2.enter_context(
                tc.tile_pool(name="attnps", bufs=2, space=bass.MemorySpace.PSUM))
            psvp = s2.enter_context(
                tc.tile_pool(name="psv", bufs=2, space=bass.MemorySpace.PSUM))

            # ---- v conv (DVE/GPSIMD; Act is saturated by exp) --------------
            convv = s1.enter_context(tc.tile_pool(name="convv", bufs=2))
            cvv = []
            for d in range(DT):
                c1 = convv.tile([128, S], BF16, name="vc1")
                nc.vector.tensor_scalar(
                    out=c1[:], in0=xpE[:, d, 2:S + 2],
                    scalar1=tap_sb["v"][:, d, 1:2], scalar2=cb_sb["v"][:, d:d + 1],
                    op0=ALU.mult, op1=ALU.add)
                t0 = convv.tile([128, S], BF16, name="vt0")
                nc.vector.tensor_scalar(
                    out=t0[:], in0=xpO[:, d, 2:S + 2],
                    scalar1=tap_sb["v"][:, d, 0:1], scalar2=None, op0=ALU.mult)
                c2 = convv.tile([128, S], BF16, name="vc2")
                nc.vector.tensor_scalar(
                    out=c2[:], in0=xpO[:, d, 4:S + 4],
                    scalar1=tap_sb["v"][:, d, 2:3], scalar2=None, op0=ALU.mult)
                eng1 = nc.gpsimd if d % 2 == 0 else nc.vector
                eng1.tensor_tensor(out=t0[:], in0=t0[:], in1=c2[:], op=ALU.add)
                cv = cvpool.tile([128, S], BF16, name=f"vcv{d}")
                nc.vector.tensor_tensor(out=cv[:], in0=t0[:], in1=c1[:], op=ALU.add)
                cvv.append(cv)

            # ---- attention helpers -----------------------------------------
            bigp = {}
            acc_of = {}

            def emit_scores(pair, chunk, ks):
                q0 = 512 * chunk
                sc = scorep.tile([128, 1024], F32, name="sc")
                for hh in range(2):
                    r0 = 64 * hh
                    nc.tensor.matmul(
                        sc[:, 512 * hh: 512 * (hh + 1)],
                        kT[r0:r0 + 64, pair, 128 * ks: 128 * (ks + 1)],
                        qT[r0:r0 + 64, pair, q0: q0 + 512],
                        start=True, stop=True, tile_position=(r0, 0))
                nc.scalar.activation(
                    bigp[(pair, chunk)][:, ks, :], sc[:], AF.Exp, scale=0.125)

            def emit_vproj(st):
                psv = psvp.tile([128, 256], F32, name="psv")
                for d in range(DT):
                    nc.tensor.matmul(
                        psv[:, 0:JL],
                        cvv[d][:, 128 * st: 128 * (st + 1)],
                        w_sb["v"][:, d, :],
                        start=(d == 0), stop=False)
                nc.tensor.matmul(
                    psv[:, 0:JL], ones_sb[0:1, :], bv2_sb[0:1, :],
                    start=False, stop=True)
                # drain on DVE (Act is saturated by exp)
                nc.vector.tensor_copy(
                    vx[:, st, :].rearrange("p (h c) -> p h c", h=4)[:, :, 0:64],
                    psv[:, 0:JL].rearrange("p (h c) -> p h c", h=4))

            def emit_attnv(pair, chunk, kp):
                # fp8 DoubleRow: k-tiles ks=2kp, 2kp+1 in one MM per hh
                bp = bigp[(pair, chunk)]
                if kp == 0:
                    acc_of[(pair, chunk)] = [
                        attnp.tile([128, 512], F32, name="acc") for _ in range(2)]
                acc = acc_of[(pair, chunk)]
                for hh in range(2):
                    hl = 2 * pair + hh
                    nc.tensor.matmul(
                        acc[hh][0:HS, :],
                        vx[:, 2 * kp:2 * kp + 2, HS * hl: HS * (hl + 1)],
                        bp[:, 2 * kp:2 * kp + 2, 512 * hh: 512 * (hh + 1)],
                        start=(kp == 0), stop=(kp == 7),
                        perf_mode=DR)

            def emit_norm(pair, chunk):
                # denominators -> reciprocal -> broadcast (gpsimd dma)
                # -> multiply (DVE); no Act involvement.
                idx = 4 * pair + chunk
                acc = acc_of.pop((pair, chunk))
                for hh in range(2):
                    dnh = nrmp.tile([1, 512], F32, name="dn", bufs=2)
                    nc.vector.tensor_copy(dnh[:], acc[hh][64:65, :])
                    nc.vector.reciprocal(dnh[:], dnh[:])
                    nc.sync.dma_start(
                        out=rdram2[2 * idx + hh: 2 * idx + hh + 1, :], in_=dnh[:])
                    nc.vector.tensor_copy(
                        attn_out[64 * hh: 64 * (hh + 1), idx, :],
                        acc[hh][0:64, :])
                bc = nrmp.tile([128, 512], F32, name="bc")
                for hh in range(2):
                    rr = rdram2[2 * idx + hh: 2 * idx + hh + 1, :]
                    bc_ap = bass.AP(
                        tensor=rr.tensor, offset=rr.offset,
                        ap=[[0, 64]] + list(rr.ap[1:]))
                    nc.gpsimd.dma_start(out=bc[64 * hh: 64 * (hh + 1), :], in_=bc_ap)
                for hh in range(2):
                    nc.vector.tensor_tensor(
                        out=attn_out[64 * hh: 64 * (hh + 1), idx, :],
                        in0=attn_out[64 * hh: 64 * (hh + 1), idx, :],
                        in1=bc[64 * hh: 64 * (hh + 1), :],
                        op=ALU.mult)

            def unit(pair, chunk, kp):
                emit_attnv(pair, chunk, kp)
                if kp == 7:
                    emit_norm(pair, chunk)

            # ---- pair-0 windows: scores + exp stream; v-proj and the
            # vx-gated attnV(0,0) threaded into windows 1..3 -----------------
            bigp[(0, 0)] = bigpA.tile([128, 16, 1024], FP8, name="bigp")
            for ks in range(16):
                emit_scores(0, 0, ks)
            bigp[(0, 1)] = bigpA.tile([128, 16, 1024], FP8, name="bigp")
            nst = 0
            for ks in range(16):
                emit_scores(0, 1, ks)
                if ks >= 7 and (ks - 7) % 3 == 0:          # vi = ks: 7, 10, 13
                    emit_vproj(nst)
                    if nst % 2 == 1:
                        unit(0, 0, (nst - 1) // 2)
                    nst += 1

            # scope-1 (x, conv transients) is done once v-conv retires;
            # recycle its SBUF for the late p tiles.
            s1.close()
            bigpB = s2.enter_context(tc.tile_pool(name="bigpB", bufs=3))

            for chunk in (2, 3):
                bigp[(0, chunk)] = bigpB.tile([128, 16, 1024], FP8, name="bigp")
                for ks in range(16):
                    emit_scores(0, chunk, ks)
                    vi = (chunk - 1) * 16 + ks
                    if (vi - 7) % 3 == 0 and nst < 14:
                        emit_vproj(nst)
                        if nst % 2 == 1:
                            unit(0, 0, (nst - 1) // 2)
                        nst += 1
            emit_vproj(14)
            emit_vproj(15)
            unit(0, 0, 7)          # kp7 + norm(0,0)

            # ---- pair-1 windows: scores + exp; deferred attnV drain --------
            bigp[(1, 0)] = bigpB.tile([128, 16, 1024], FP8, name="bigp")
            for ks in range(16):
                emit_scores(1, 0, ks)
                unit(0, 1, ks) if ks < 8 else unit(0, 2, ks - 8)
            bigp[(1, 1)] = bigpA.tile([128, 16, 1024], FP8, name="bigp")
            for ks in range(16):
                emit_scores(1, 1, ks)
                unit(0, 3, ks) if ks < 8 else unit(1, 0, ks - 8)
            bigp[(1, 2)] = bigpA.tile([128, 16, 1024], FP8, name="bigp")
            for ks in range(16):
                emit_scores(1, 2, ks)
                if ks < 8:
                    unit(1, 1, ks)
                elif ks >= 9:
                    unit(1, 2, ks - 9)             # kp0-6, trailing exp by 1+
            bigp[(1, 3)] = bigpB.tile([128, 16, 1024], FP8, name="bigp")
            for ks in range(16):
                emit_scores(1, 3, ks)
                if ks == 0:
                    unit(1, 2, 7)
                elif ks >= 2 and ks % 2 == 0:
                    unit(1, 3, (ks - 2) // 2)      # kp0-6
            unit(1, 3, 7)

            # close attention pools (PSUM) before the output projection
            s2.close()

            # ================= phase D: output projection ====================
            # chunk-major with 1-bank PSUM tiles; drains cycle DVE/Act/GPSIMD
            with tc.tile_pool(name="psum_o", bufs=2, space=bass.MemorySpace.PSUM) as psum_o, \
                 tc.tile_pool(name="ypool", bufs=3) as ypool:
                for chunk in range(4):
                    for m in range(8):
                        ps = psum_o.tile([128, 512], F32, name="ps_o")
                        for pair in range(2):
                            nc.tensor.matmul(
                                ps[:, :],
                                wo_sb[:, pair, 128 * m: 128 * (m + 1)],
                                attn_out[:, 4 * pair + chunk, :],
                                start=(pair == 0), stop=(pair == 1))
                        yt = ypool.tile([128, 512], F32, name="yt")
                        if (chunk * 8 + m) % 2 == 0:
                            nc.vector.tensor_copy(yt[:], ps[:])
                        else:
                            nc.scalar.copy(yt[:], ps[:])
                        nc.sync.dma_start(
                            out=P['y'][128 * m: 128 * (m + 1), 512 * chunk: 512 * (chunk + 1)],
                            in_=yt[:])

    split_multi_waits(nc)
    return nc


# ---------------------------------------------------------------------------
def make_in_maps(x, dwq_w, dwq_b, dwk_w, dwk_b, dwv_w, dwv_b,
                 wq, bq, wk, bk, wv, bv, wo, bo):
    bf = ml_dtypes.bfloat16
    in_maps = []
    xp_cache = {}
    for c in range(N_CORES):
        b, g = divmod(c, 4)
        js = slice(JL * g, JL * (g + 1))
        if b not in xp_cache:
            xE = np.zeros((D, S + 4), np.float32)
            xE[:, 2:S + 2] = x[b].T
            xO = np.zeros((D, S + 4), np.float32)
            xO[:, 3:S + 3] = x[b].T
            xp_cache[b] = (
                np.ascontiguousarray(xE.reshape(DT, 128, S + 4).transpose(1, 0, 2)).astype(bf),
                np.ascontiguousarray(xO.reshape(DT, 128, S + 4).transpose(1, 0, 2)).astype(bf))
        m = {'xpE': xp_cache[b][0], 'xpO': xp_cache[b][1]}
        for t, w_, dw_w, dw_b, pb_ in (("q", wq, dwq_w, dwq_b, bq),
                                       ("k", wk, dwk_w, dwk_b, bk),
                                       ("v", wv, dwv_w, dwv_b, bv)):
            m['w' + t] = np.ascontiguousarray(
                w_[js, :].T.reshape(DT, 128, JL).transpose(1, 0, 2)).astype(bf)
            m['tap' + t] = np.ascontiguousarray(
                dw_w.reshape(DT, 128, 3).transpose(1, 0, 2)).astype(np.float32)
            m['cb' + t] = np.ascontiguousarray(dw_b.reshape(DT, 128).T).astype(np.float32)
            if t in ("q", "k"):
                m['pb' + t] = np.ascontiguousarray(pb_[js].reshape(2, 128).T).astype(np.float32)
        m['bv2'] = bv[js].reshape(1, JL).astype(bf)
        m['wo'] = np.ascontiguousarray(
            wo[:, js].T.reshape(2, 128, D).transpose(1, 0, 2)).astype(bf)
        in_maps.append(m)
    return in_maps


def gather_output(results, bo):
    B = 2
    out = np.zeros((B, S, D), np.float32)
    for c in range(N_CORES):
        b = c // 4
        out[b] += results[c]['y'].T
    out += bo
    return out


# ---------------------------------------------------------------------------
_PROGRAM_CACHE = {}


def kernel(x, dwq_w, dwq_b, dwk_w, dwk_b, dwv_w, dwv_b,
           wq, bq, wk, bk, wv, bv, wo, bo):
    """Full-input entry point: shards across 8 NeuronCores internally."""
    from concourse.bass_utils import run_bass_kernel_spmd

    x = np.asarray(x, np.float32)
    args = dict(x=x,
                dwq_w=np.asarray(dwq_w, np.float32), dwq_b=np.asarray(dwq_b, np.float32),
                dwk_w=np.asarray(dwk_w, np.float32), dwk_b=np.asarray(dwk_b, np.float32),
                dwv_w=np.asarray(dwv_w, np.float32), dwv_b=np.asarray(dwv_b, np.float32),
                wq=np.asarray(wq, np.float32), bq=np.asarray(bq, np.float32),
                wk=np.asarray(wk, np.float32), bk=np.asarray(bk, np.float32),
                wv=np.asarray(wv, np.float32), bv=np.asarray(bv, np.float32),
                wo=np.asarray(wo, np.float32), bo=np.asarray(bo, np.float32))
    if 'nc' not in _PROGRAM_CACHE:
        _PROGRAM_CACHE['nc'] = build_program()
    nc = _PROGRAM_CACHE['nc']
    in_maps = make_in_maps(**args)
    res = run_bass_kernel_spmd(nc, in_maps, list(range(N_CORES)))
    return gather_output(res.results, args['bo']).astype(np.float32)


# revision 16
# speedup vs baseline: 1.2372x; 1.2372x over previous
"""DepthwiseSeparableAttention Trainium2 kernel (8-core SPMD).

Sharding: core c -> (batch b = c//4, head-group g = c%4, 4 heads each).
Each core computes depthwise-conv + QKV projection for its head slice,
attention for its 4 heads, and a partial output projection; the host sums
the 4 partials per batch and adds the output bias.

Schedule (v2): the scalar-engine exp stream (128 x [128,1024] EXPs ~= 128us)
is the critical resource.  The front is organised to start it ASAP (~50us):
x d-tile 0 ships first, the conv is split across DVE/GPSIMD/Act, q/k
projections use 2048-wide compound matmuls (q then k sequentially to fit
PSUM).  v conv + v projection, attention-V (fp8 DoubleRow over persisted
fp8 p slabs), softmax normalization and the output projection all run
under / after the exp stream, threaded into its windows so the Act engine
never stalls.
"""
import os
import sys
for _p in ('/opt/trn_rl_repo', '/root/.axon_site/_ro/trn_rl_repo'):
    if os.path.isdir(_p):
        sys.path.insert(0, _p)
        break

import numpy as np
import ml_dtypes

import concourse.bass as bass
import concourse.mybir as mybir
import concourse.tile as tile
from concourse.vector_clock import ScopedClock

BF16 = mybir.dt.bfloat16
F32 = mybir.dt.float32
FP8 = mybir.dt.float8e4
AF = mybir.ActivationFunctionType
ALU = mybir.AluOpType
DR = mybir.MatmulPerfMode.DoubleRow

S = 2048          # sequence length
D = 1024          # model dim
DT = 8            # d-tiles of 128
JL = 256          # local head channels (4 heads x 64)
HS = 80           # per-head stride in vx (64 d + ones col + pad; 16B-aligned)
N_CORES = 8

# ---------------------------------------------------------------------------
# walrus in this env allows only ONE sync wait per instruction; split Tile's
# excess waits onto no-fuse NOPs / extra drains.
MAX_WAITS = 1


def _patched_drain_and_barrier(self, tick_clock, wait_clock):
    drain_inst = self.nc.sync.drain()
    wait_clock.add_sem_waits(drain_inst.ins, ScopedClock({None: tick_clock.global_clock}))
    si = drain_inst.ins.sync_info
    if si is not None and len(si.on_wait) > 1:
        waits = list(si.on_wait)
        drain_inst.ins.sync_info = mybir.SyncInfo(on_wait=[waits[0]], on_update=list(si.on_update))
        for w in waits[1:]:
            d2 = self.nc.sync.drain()
            d2.ins.sync_info = mybir.SyncInfo(on_wait=[w], on_update=[])
    self.nc.all_engine_barrier()
    popped = self.nc._tile_sem_poison_stack.pop()
    assert popped is self._sem_poison
    self.nc.clear_and_free_semaphores(list(self.sems.allocated().values()))
    self.nc.all_engine_barrier()


tile.TileContext._drain_and_barrier = _patched_drain_and_barrier


def split_multi_waits(nc):
    n_split = 0
    for f in nc.m.functions:
        for blk in f.blocks:
            il = blk.instructions
            if not any(i.sync_info and len(i.sync_info.on_wait) > MAX_WAITS for i in il):
                continue
            newlist = []
            for inst in il:
                si = inst.sync_info
                if si is not None and len(si.on_wait) > MAX_WAITS:
                    waits = list(si.on_wait)
                    head, tail = waits[:-MAX_WAITS], waits[-MAX_WAITS:]
                    for j, w in enumerate(head):
                        nop = mybir.InstNoOp(
                            name=f"{inst.name}-w{j}",
                            sync_info=mybir.SyncInfo(on_wait=[w], on_update=[]),
                            bass_nofuse=True,
                            engine=inst.engine,
                        )
                        newlist.append(nop)
                        n_split += 1
                    inst.sync_info = mybir.SyncInfo(on_wait=tail, on_update=list(si.on_update))
                newlist.append(inst)
            blk.instructions = newlist
    return n_split


# ---------------------------------------------------------------------------
def build_program(n_rep=1):
    import contextlib
    nc = bass.Bass()
    P = {}
    P['xpE'] = nc.declare_dram_parameter("xpE", [128, DT, S + 4], BF16, isOutput=False)
    P['xpO'] = nc.declare_dram_parameter("xpO", [128, DT, S + 4], BF16, isOutput=False)
    for t in ("q", "k", "v"):
        P['w' + t] = nc.declare_dram_parameter("w" + t, [128, DT, JL], BF16, isOutput=False)
        P['tap' + t] = nc.declare_dram_parameter("tap" + t, [128, DT, 3], F32, isOutput=False)
        P['cb' + t] = nc.declare_dram_parameter("cb" + t, [128, DT], F32, isOutput=False)
    P['pbq'] = nc.declare_dram_parameter("pbq", [128, 2], F32, isOutput=False)
    P['pbk'] = nc.declare_dram_parameter("pbk", [128, 2], F32, isOutput=False)
    P['bv2'] = nc.declare_dram_parameter("bv2", [1, JL], BF16, isOutput=False)
    P['wo'] = nc.declare_dram_parameter("wo", [128, 2, D], BF16, isOutput=False)
    P['y'] = nc.declare_dram_parameter("y", [D, S], F32, isOutput=True)
    denom_dram = nc.dram_tensor("denom_scratch", [16, 512], F32)
    rdram2 = nc.dram_tensor("recip_scratch2", [16, 512], F32)

    with tile.TileContext(nc) as tc:
        with contextlib.ExitStack() as ctx:
            # outer pools: live for the whole kernel
            consts = ctx.enter_context(tc.tile_pool(name="consts", bufs=1))
            qkvp = ctx.enter_context(tc.tile_pool(name="qkvp", bufs=1))
            cvpool = ctx.enter_context(tc.tile_pool(name="cvpool", bufs=1))
            bigpA = ctx.enter_context(tc.tile_pool(name="bigpA", bufs=2))
            nrmp = ctx.enter_context(tc.tile_pool(name="nrm", bufs=1))

            # scope-1: x + conv transients; closed mid-attention so the late
            # p tiles (bigpB) can reuse the SBUF.
            s1 = ctx.enter_context(contextlib.ExitStack())
            xpool = s1.enter_context(tc.tile_pool(name="xpool", bufs=1))

            # ---- constants + input, DMA'd in consumption order -------------
            tap_sb, cb_sb, w_sb, pb_sb = {}, {}, {}, {}
            for t in ("q", "k"):
                tap_sb[t] = consts.tile([128, DT, 3], F32, name="tap_" + t)
                nc.sync.dma_start(out=tap_sb[t][:], in_=P['tap' + t][:])
                cb_sb[t] = consts.tile([128, DT], F32, name="cb_" + t)
                nc.sync.dma_start(out=cb_sb[t][:], in_=P['cb' + t][:])
            xpE = xpool.tile([128, DT, S + 4], BF16, name="xpE")
            xpO = xpool.tile([128, DT, S + 4], BF16, name="xpO")
            nc.sync.dma_start(out=xpE[:, 0, :], in_=P['xpE'][:, 0, :])
            nc.sync.dma_start(out=xpO[:, 0, :], in_=P['xpO'][:, 0, :])
            for t in ("q", "k"):
                w_sb[t] = consts.tile([128, DT, JL], BF16, name="w_" + t)
                nc.sync.dma_start(out=w_sb[t][:], in_=P['w' + t][:])
            nc.sync.dma_start(out=xpE[:, 1, :], in_=P['xpE'][:, 1, :])
            nc.sync.dma_start(out=xpO[:, 1, :], in_=P['xpO'][:, 1, :])
            for t in ("q", "k"):
                pb_sb[t] = consts.tile([128, 2], F32, name="pb_" + t)
                nc.sync.dma_start(out=pb_sb[t][:], in_=P['pb' + t][:])
            for d in range(2, DT):
                nc.sync.dma_start(out=xpE[:, d, :], in_=P['xpE'][:, d, :])
                nc.sync.dma_start(out=xpO[:, d, :], in_=P['xpO'][:, d, :])
            tap_sb["v"] = consts.tile([128, DT, 3], F32, name="tap_v")
            nc.sync.dma_start(out=tap_sb["v"][:], in_=P['tapv'][:])
            cb_sb["v"] = consts.tile([128, DT], F32, name="cb_v")
            nc.sync.dma_start(out=cb_sb["v"][:], in_=P['cbv'][:])
            w_sb["v"] = consts.tile([128, DT, JL], BF16, name="w_v")
            nc.sync.dma_start(out=w_sb["v"][:], in_=P['wv'][:])
            bv2_sb = consts.tile([1, JL], BF16)
            nc.sync.dma_start(out=bv2_sb[:], in_=P['bv2'][:])
            wo_sb = consts.tile([128, 2, D], BF16)
            nc.sync.dma_start(out=wo_sb[:], in_=P['wo'][:])
            ones_sb = consts.tile([1, 128], BF16)
            nc.vector.memset(ones_sb[:], 1.0)

            # ---- persistent activations -----------------------------------
            qT = qkvp.tile([128, 2, S], BF16, name="qT")      # [j_in_tile, j_tile, s]
            kT = qkvp.tile([128, 2, S], BF16, name="kT")
            # v in fp8, [s_in_tile, s_tile, head*HS]: col 64 of each head
            # block is the softmax-denominator ones column, 65..79 pad
            # (HS=80 keeps the DoubleRow weight k-tile step 16B-aligned).
            vx = qkvp.tile([128, 16, 4 * HS], FP8, name="vx")
            for h in range(4):
                nc.vector.memset(vx[:, :, HS * h + 64: HS * h + 65], 1.0)
                nc.vector.memset(vx[:, :, HS * h + 65: HS * (h + 1)], 0.0)
            attn_out = qkvp.tile([128, 8, 512], BF16, name="attn_out")

            # ================= phase B: conv + QK projections ================
            # Split-stream conv: the PE combines the two partial streams in
            # PSUM (doubling qk contraction), keeping the DVE chain short:
            #   cv = mid*x + bias   (Act, feeds the PE directly)
            #   c2 = left + right   (DVE: TS, TS, add)
            # q fully, then k (their two 4-bank PSUM pairs don't fit at once).
            with tc.tile_pool(name="convqk", bufs=2) as convqk, \
                 tc.tile_pool(name="psqk", bufs=2, space=bass.MemorySpace.PSUM) as psqk:
                for t, dst in (("q", qT), ("k", kT)):
                    ps = [psqk.tile([128, S], F32, name="ps_qk") for _ in range(2)]
                    for d in range(DT):
                        cv = convqk.tile([128, S], BF16, name="cv")
                        nc.scalar.activation(
                            cv[:], xpE[:, d, 2:S + 2], AF.Identity,
                            bias=cb_sb[t][:, d:d + 1], scale=tap_sb[t][:, d, 1:2])
                        t0 = convqk.tile([128, S], BF16, name="t0")
                        nc.vector.tensor_scalar(
                            out=t0[:], in0=xpO[:, d, 2:S + 2],
                            scalar1=tap_sb[t][:, d, 0:1], scalar2=None, op0=ALU.mult)
                        c2 = convqk.tile([128, S], BF16, name="c2")
                        nc.vector.tensor_scalar(
                            out=c2[:], in0=xpO[:, d, 4:S + 4],
                            scalar1=tap_sb[t][:, d, 2:3], scalar2=None, op0=ALU.mult)
                        nc.vector.tensor_tensor(out=c2[:], in0=c2[:], in1=t0[:], op=ALU.add)
                        for m in range(2):
                            for s, src in enumerate((cv, c2)):
                                for c in range(4):
                                    nc.tensor.matmul(
                                        ps[m][:, 512 * c: 512 * (c + 1)],
                                        w_sb[t][:, d, 128 * m: 128 * (m + 1)],
                                        src[:, 512 * c: 512 * (c + 1)],
                                        start=(d == 0 and s == 0),
                                        stop=(d == DT - 1 and s == 1))
                    # all four drains on Act (DVE moves straight to v conv)
                    for m in range(2):
                        nc.scalar.activation(
                            dst[:, m, :], ps[m][:], AF.Identity,
                            bias=pb_sb[t][:, m: m + 1], scale=1.0)

            # ================= phase C: attention (+ v under it) =============
            # PSUM: scores 3x2 banks + (v-proj 2x1 during pair-0 windows,
            # then attn acc 2x1 from w4 on - their scopes are sequential).
            s2 = ctx.enter_context(contextlib.ExitStack())
            scorep = s2.enter_context(
                tc.tile_pool(name="scores", bufs=3, space=bass.MemorySpace.PSUM))
            sv = contextlib.ExitStack()
            psvp = sv.enter_context(
                tc.tile_pool(name="psv", bufs=2, space=bass.MemorySpace.PSUM))

            # ---- v conv (DVE/GPSIMD; Act is saturated by exp) --------------
            convv = s1.enter_context(tc.tile_pool(name="convv", bufs=2))
            cvv = []
            for d in range(DT):
                c1 = convv.tile([128, S], BF16, name="vc1")
                nc.vector.tensor_scalar(
                    out=c1[:], in0=xpE[:, d, 2:S + 2],
                    scalar1=tap_sb["v"][:, d, 1:2], scalar2=cb_sb["v"][:, d:d + 1],
                    op0=ALU.mult, op1=ALU.add)
                t0 = convv.tile([128, S], BF16, name="vt0")
                nc.vector.tensor_scalar(
                    out=t0[:], in0=xpO[:, d, 2:S + 2],
                    scalar1=tap_sb["v"][:, d, 0:1], scalar2=None, op0=ALU.mult)
                c2 = convv.tile([128, S], BF16, name="vc2")
                nc.vector.tensor_scalar(
                    out=c2[:], in0=xpO[:, d, 4:S + 4],
                    scalar1=tap_sb["v"][:, d, 2:3], scalar2=None, op0=ALU.mult)
                # GPSIMD takes half the first adds (not latency-critical)
                eng1 = nc.gpsimd if d % 2 == 1 else nc.vector
                eng1.tensor_tensor(out=t0[:], in0=t0[:], in1=c2[:], op=ALU.add)
                cv = cvpool.tile([128, S], BF16, name=f"vcv{d}")
                nc.vector.tensor_tensor(out=cv[:], in0=t0[:], in1=c1[:], op=ALU.add)
                cvv.append(cv)

            # ---- attention helpers -----------------------------------------
            bigp = {}
            acc_of = {}

            def emit_scores(pair, chunk, ks):
                q0 = 512 * chunk
                sc = scorep.tile([128, 1024], F32, name="sc")
                for hh in range(2):
                    r0 = 64 * hh
                    nc.tensor.matmul(
                        sc[:, 512 * hh: 512 * (hh + 1)],
                        kT[r0:r0 + 64, pair, 128 * ks: 128 * (ks + 1)],
                        qT[r0:r0 + 64, pair, q0: q0 + 512],
                        start=True, stop=True, tile_position=(r0, 0))
                nc.scalar.activation(
                    bigp[(pair, chunk)][:, ks, :], sc[:], AF.Exp, scale=0.125)

            def emit_vproj(st):
                psv = psvp.tile([128, 256], F32, name="psv")
                for d in range(DT):
                    nc.tensor.matmul(
                        psv[:, 0:JL],
                        cvv[d][:, 128 * st: 128 * (st + 1)],
                        w_sb["v"][:, d, :],
                        start=(d == 0), stop=False)
                nc.tensor.matmul(
                    psv[:, 0:JL], ones_sb[0:1, :], bv2_sb[0:1, :],
                    start=False, stop=True)
                # drain on DVE (Act is saturated by exp)
                nc.vector.tensor_copy(
                    vx[:, st, :].rearrange("p (h c) -> p h c", h=4)[:, :, 0:64],
                    psv[:, 0:JL].rearrange("p (h c) -> p h c", h=4))

            def emit_attnv(pair, chunk, kp):
                # fp8 DoubleRow: k-tiles ks=2kp, 2kp+1 in one MM per hh
                bp = bigp[(pair, chunk)]
                if kp == 0:
                    acc_of[(pair, chunk)] = [
                        attnp.tile([128, 512], F32, name="acc") for _ in range(2)]
                acc = acc_of[(pair, chunk)]
                for hh in range(2):
                    hl = 2 * pair + hh
                    nc.tensor.matmul(
                        acc[hh][0:HS, :],
                        vx[:, 2 * kp:2 * kp + 2, HS * hl: HS * (hl + 1)],
                        bp[:, 2 * kp:2 * kp + 2, 512 * hh: 512 * (hh + 1)],
                        start=(kp == 0), stop=(kp == 7),
                        perf_mode=DR)

            def emit_stash(pair, chunk):
                # stash unnormalized output + per-head denominator rows
                idx = 4 * pair + chunk
                acc = acc_of.pop((pair, chunk))
                for hh in range(2):
                    dnh = nrmp.tile([1, 512], F32, name="dn", bufs=2)
                    nc.vector.tensor_copy(dnh[:], acc[hh][64:65, :])
                    nc.sync.dma_start(
                        out=denom_dram[2 * idx + hh: 2 * idx + hh + 1, :], in_=dnh[:])
                    nc.vector.tensor_copy(
                        attn_out[64 * hh: 64 * (hh + 1), idx, :],
                        acc[hh][0:64, :])

            def emit_pair_norm(pair):
                # one batched reciprocal for the pair's 8 denominator rows,
                # then per-chunk broadcast (gpsimd dma) + multiply (DVE).
                dn8 = nrmp.tile([8, 512], F32, name="dn8", bufs=1)
                nc.sync.dma_start(out=dn8[:], in_=denom_dram[8 * pair: 8 * pair + 8, :])
                nc.vector.reciprocal(dn8[:], dn8[:])
                nc.sync.dma_start(out=rdram2[8 * pair: 8 * pair + 8, :], in_=dn8[:])
                for chunk in range(4):
                    idx = 4 * pair + chunk
                    bc = nrmp.tile([128, 512], F32, name="bc", bufs=1)
                    for hh in range(2):
                        rr = rdram2[2 * idx + hh: 2 * idx + hh + 1, :]
                        bc_ap = bass.AP(
                            tensor=rr.tensor, offset=rr.offset,
                            ap=[[0, 64]] + list(rr.ap[1:]))
                        nc.gpsimd.dma_start(out=bc[64 * hh: 64 * (hh + 1), :], in_=bc_ap)
                    for hh in range(2):
                        nc.vector.tensor_tensor(
                            out=attn_out[64 * hh: 64 * (hh + 1), idx, :],
                            in0=attn_out[64 * hh: 64 * (hh + 1), idx, :],
                            in1=bc[64 * hh: 64 * (hh + 1), :],
                            op=ALU.mult)

            def unit(pair, chunk, kp):
                emit_attnv(pair, chunk, kp)
                if kp == 7:
                    emit_stash(pair, chunk)

            # ---- pair-0 windows: scores + exp stream; v-proj threaded into
            # windows 1..3 (all attnV deferred to the pair-1 windows) --------
            bigp[(0, 0)] = bigpA.tile([128, 16, 1024], FP8, name="bigp")
            for ks in range(16):
                emit_scores(0, 0, ks)
            bigp[(0, 1)] = bigpA.tile([128, 16, 1024], FP8, name="bigp")
            nst = 0
            for ks in range(16):
                emit_scores(0, 1, ks)
                if ks >= 7 and (ks - 7) % 3 == 0:          # ks: 7, 10, 13
                    emit_vproj(nst)
                    nst += 1

            # scope-1 (x, conv transients) is done once v-conv retires;
            # recycle its SBUF for the late p tiles.
            s1.close()
            bigpB = s2.enter_context(tc.tile_pool(name="bigpB", bufs=3))

            for chunk in (2, 3):
                bigp[(0, chunk)] = bigpB.tile([128, 16, 1024], FP8, name="bigp")
                for ks in range(16):
                    emit_scores(0, chunk, ks)
                    vi = (chunk - 1) * 16 + ks
                    if (vi - 7) % 3 == 0 and nst < 14:
                        emit_vproj(nst)
                        nst += 1
            emit_vproj(14)
            emit_vproj(15)

            # v-proj PSUM scope ends; attnV accumulators take its banks
            sv.close()
            attnp = s2.enter_context(
                tc.tile_pool(name="attnps", bufs=2, space=bass.MemorySpace.PSUM))

            # ---- pair-1 windows: scores + exp; deferred attnV drain at
            # 1 kp-unit per ks slot ------------------------------------------
            bigp[(1, 0)] = bigpB.tile([128, 16, 1024], FP8, name="bigp")
            for ks in range(16):
                emit_scores(1, 0, ks)
                unit(0, 0, ks) if ks < 8 else unit(0, 1, ks - 8)
            bigp[(1, 1)] = bigpA.tile([128, 16, 1024], FP8, name="bigp")
            for ks in range(16):
                emit_scores(1, 1, ks)
                unit(0, 2, ks) if ks < 8 else unit(0, 3, ks - 8)
            emit_pair_norm(0)
            bigp[(1, 2)] = bigpA.tile([128, 16, 1024], FP8, name="bigp")
            for ks in range(16):
                emit_scores(1, 2, ks)
                unit(1, 0, ks) if ks < 8 else unit(1, 1, ks - 8)
            bigp[(1, 3)] = bigpB.tile([128, 16, 1024], FP8, name="bigp")
            for ks in range(16):
                emit_scores(1, 3, ks)
                if ks < 8:
                    unit(1, 2, ks)
                elif ks >= 10:
                    unit(1, 3, ks - 10)            # kp0-5, trailing exps
            unit(1, 3, 6)
            unit(1, 3, 7)
            emit_pair_norm(1)

            # close attention pools (PSUM) before the output projection
            s2.close()

            # ================= phase D: output projection ====================
            # chunk-major with 1-bank PSUM tiles; drains cycle DVE/Act/GPSIMD
            with tc.tile_pool(name="psum_o", bufs=2, space=bass.MemorySpace.PSUM) as psum_o, \
                 tc.tile_pool(name="ypool", bufs=3) as ypool:
                for chunk in range(4):
                    for m in range(8):
                        ps = psum_o.tile([128, 512], F32, name="ps_o")
                        for pair in range(2):
                            nc.tensor.matmul(
                                ps[:, :],
                                wo_sb[:, pair, 128 * m: 128 * (m + 1)],
                                attn_out[:, 4 * pair + chunk, :],
                                start=(pair == 0), stop=(pair == 1))
                        yt = ypool.tile([128, 512], F32, name="yt")
                        if (chunk * 8 + m) % 2 == 0:
                            nc.vector.tensor_copy(yt[:], ps[:])
                        else:
                            nc.scalar.copy(yt[:], ps[:])
                        nc.sync.dma_start(
                            out=P['y'][128 * m: 128 * (m + 1), 512 * chunk: 512 * (chunk + 1)],
                            in_=yt[:])

    split_multi_waits(nc)
    return nc


# ---------------------------------------------------------------------------
def make_in_maps(x, dwq_w, dwq_b, dwk_w, dwk_b, dwv_w, dwv_b,
                 wq, bq, wk, bk, wv, bv, wo, bo):
    bf = ml_dtypes.bfloat16
    in_maps = []
    xp_cache = {}
    for c in range(N_CORES):
        b, g = divmod(c, 4)
        js = slice(JL * g, JL * (g + 1))
        if b not in xp_cache:
            xE = np.zeros((D, S + 4), np.float32)
            xE[:, 2:S + 2] = x[b].T
            xO = np.zeros((D, S + 4), np.float32)
            xO[:, 3:S + 3] = x[b].T
            xp_cache[b] = (
                np.ascontiguousarray(xE.reshape(DT, 128, S + 4).transpose(1, 0, 2)).astype(bf),
                np.ascontiguousarray(xO.reshape(DT, 128, S + 4).transpose(1, 0, 2)).astype(bf))
        m = {'xpE': xp_cache[b][0], 'xpO': xp_cache[b][1]}
        for t, w_, dw_w, dw_b, pb_ in (("q", wq, dwq_w, dwq_b, bq),
                                       ("k", wk, dwk_w, dwk_b, bk),
                                       ("v", wv, dwv_w, dwv_b, bv)):
            m['w' + t] = np.ascontiguousarray(
                w_[js, :].T.reshape(DT, 128, JL).transpose(1, 0, 2)).astype(bf)
            m['tap' + t] = np.ascontiguousarray(
                dw_w.reshape(DT, 128, 3).transpose(1, 0, 2)).astype(np.float32)
            m['cb' + t] = np.ascontiguousarray(dw_b.reshape(DT, 128).T).astype(np.float32)
            if t in ("q", "k"):
                m['pb' + t] = np.ascontiguousarray(pb_[js].reshape(2, 128).T).astype(np.float32)
        m['bv2'] = bv[js].reshape(1, JL).astype(bf)
        m['wo'] = np.ascontiguousarray(
            wo[:, js].T.reshape(2, 128, D).transpose(1, 0, 2)).astype(bf)
        in_maps.append(m)
    return in_maps


def gather_output(results, bo):
    B = 2
    out = np.zeros((B, S, D), np.float32)
    for c in range(N_CORES):
        b = c // 4
        out[b] += results[c]['y'].T
    out += bo
    return out


# ---------------------------------------------------------------------------
_PROGRAM_CACHE = {}


def kernel(x, dwq_w, dwq_b, dwk_w, dwk_b, dwv_w, dwv_b,
           wq, bq, wk, bk, wv, bv, wo, bo):
    """Full-input entry point: shards across 8 NeuronCores internally."""
    from concourse.bass_utils import run_bass_kernel_spmd

    x = np.asarray(x, np.float32)
    args = dict(x=x,
                dwq_w=np.asarray(dwq_w, np.float32), dwq_b=np.asarray(dwq_b, np.float32),
                dwk_w=np.asarray(dwk_w, np.float32), dwk_b=np.asarray(dwk_b, np.float32),
                dwv_w=np.asarray(dwv_w, np.float32), dwv_b=np.asarray(dwv_b, np.float32),
                wq=np.asarray(wq, np.float32), bq=np.asarray(bq, np.float32),
                wk=np.asarray(wk, np.float32), bk=np.asarray(bk, np.float32),
                wv=np.asarray(wv, np.float32), bv=np.asarray(bv, np.float32),
                wo=np.asarray(wo, np.float32), bo=np.asarray(bo, np.float32))
    if 'nc' not in _PROGRAM_CACHE:
        _PROGRAM_CACHE['nc'] = build_program()
    nc = _PROGRAM_CACHE['nc']
    in_maps = make_in_maps(**args)
    res = run_bass_kernel_spmd(nc, in_maps, list(range(N_CORES)))
    return gather_output(res.results, args['bo']).astype(np.float32)
